# revision 1
# baseline (speedup 1.0000x reference)
"""FFT spatially-variant blur as direct separable convolution on 8 trn2 cores.

Math: reference blurs image with 8 Gaussian PSF bases via FFT, then mixes
per-pixel with weights w_k = exp(-(sigma-s_k)^2/2) (normalized over k),
sigma = clip(softplus(0.3*coc+0.5), 0.2, 12).  With coc in [0,1),
sigma in [0.974, 1.172], so normalized weights for k>=4 are < 5e-8 ->
below fp32 noise; only bases k=0..3 contribute.

Each Gaussian PSF separates into an outer product of 1D taps, so
blur_k = T_k^T @ X @ T_k with T_k a banded (31-diag) Toeplitz matrix.
Both stages run on the tensor engine with the image/intermediate as the
stationary operand and T_k as the moving operand (zero transposes):
  stage 1: A^T = lhsT(X).T @ T_k      (column conv, transposed result)
  stage 2: Z   = lhsT(A^T).T @ T_k    (row conv, natural result)
Banded structure -> matmuls restricted to N-windows near the diagonal.

Data parallel: core b handles batch sample b.
"""

import numpy as np

PSF_SIZE = 31
SIGMA_MIN = 0.2
SIGMA_MAX = 12.0
EPS = 1e-9
NUM_BASES_USED = 4
H = 512
NCHUNK = 4  # 512 / 128

MODE = "f32r"  # "f32r" | "bf16" | "f32"

# stage-1 N-windows: (col0, width, [contributing row-chunks])
# band of chunk q covers cols [128q-15, 128q+143); overlap regions must get
# matmuls from both chunks.
_WINDOWS_S1 = {
    # >=64 wide so k-packed width*4 >= 256 (f32r full-rate threshold)
    "f32r": [
        (0, 96, (0,)),
        (96, 64, (0, 1)),
        (160, 64, (1,)),
        (224, 64, (1, 2)),
        (288, 64, (2,)),
        (352, 64, (2, 3)),
        (416, 96, (3,)),
    ],
    # exact band windows (smallest streamed N)
    "bf16": [
        (0, 113, (0,)),
        (113, 30, (0, 1)),
        (143, 98, (1,)),
        (241, 30, (1, 2)),
        (271, 98, (2,)),
        (369, 30, (2, 3)),
        (399, 113, (3,)),
    ],
}
_WINDOWS_S1["f32"] = _WINDOWS_S1["bf16"]

# stage-1 PSUM bank packing: bank -> list of window indices (k-packed widths
# per bank must total <= 512 fp32)
_BANKS_S1 = [[0], [1, 2], [3, 4], [5], [6]]

# stage 2 (T-stationary, transposed out): per c'-tile ct, contraction over
# band chunks q' in {ct-1, ct, ct+1}
_S2_CHUNKS = [tuple(q for q in (ct - 1, ct, ct + 1) if 0 <= q < NCHUNK)
              for ct in range(NCHUNK)]


def _taps():
    """Normalized 1D tap vectors per basis, fp32.  outer(t,t) == 2D psf."""
    lo = (-PSF_SIZE) // 2
    hi = PSF_SIZE // 2
    x = np.linspace(lo, hi, PSF_SIZE, dtype=np.float32).astype(np.float64)
    sigmas = np.linspace(SIGMA_MIN, SIGMA_MAX, 8, dtype=np.float32)
    out = []
    for k in range(NUM_BASES_USED):
        f = np.exp(-(x ** 2) / (2.0 * float(sigmas[k]) ** 2 + EPS))
        fn = f / np.sqrt(f.sum() ** 2 + EPS)
        out.append(fn.astype(np.float32))
    return out, sigmas


def _softplus_poly(ws, bs):
    """Degree-4 fit of softplus(ws*c + bs) on c in [-0.01, 1.01].
    Returns [g3, g2, g1, g0, a4] for Horner ((((c+g3)c+g2)c+g1)c+g0)*a4."""
    c = np.linspace(-0.01, 1.01, 4001)
    y = np.logaddexp(0.0, ws * c + bs)
    a = np.polyfit(c, y, 4)  # a[0]=a4 ... a[4]=a0
    a4 = a[0] if abs(a[0]) > 1e-30 else 1e-30
    return np.array([a[1] / a4, a[2] / a4, a[3] / a4, a[4] / a4, a4],
                    dtype=np.float32)


# compact band column ranges per chunk (width 160 covers the 158-wide band)
_BAND_C0 = [0, 113, 241, 352]
_BAND_W = 160


def _taps_padded():
    """Compact band table [4 (q), 128, 4k*160] fp32: only the nonzero
    diagonal band of each Toeplitz chunk T_k[m, j] = taps[15-m+j]; the
    rest of the on-device tile is memset to zero."""
    taps, _ = _taps()
    tab = np.zeros((NCHUNK, 128, NUM_BASES_USED * _BAND_W), dtype=np.float32)
    for k in range(NUM_BASES_USED):
        Tm = np.zeros((H, H), dtype=np.float32)
        for m in range(H):
            j0 = max(0, m - 15)
            j1 = min(H, m + 16)
            Tm[m, j0:j1] = taps[k][15 - (m - np.arange(j0, j1))]
        for q in range(NCHUNK):
            c0 = _BAND_C0[q]
            tab[q, :, k * _BAND_W:(k + 1) * _BAND_W] = \
                Tm[q * 128:(q + 1) * 128, c0:c0 + _BAND_W]
    return tab


def _build(mode):
    import concourse.bass as bass  # noqa: F401
    import concourse.tile as tile
    from concourse import mybir, bacc

    f32 = mybir.dt.float32
    DT = {"f32r": mybir.dt.float32r, "bf16": mybir.dt.bfloat16,
          "f32": mybir.dt.float32}[mode]
    AF = mybir.ActivationFunctionType
    ALU = mybir.AluOpType
    K = NUM_BASES_USED
    wins1 = _WINDOWS_S1[mode]
    _, sigmas = _taps()

    nc = bacc.Bacc("TRN2", target_bir_lowering=False, debug=False,
                   disable_frame_to_traceback=True)
    IMG = nc.declare_dram_parameter("image", [3, H, H], f32, isOutput=False)
    # coc TRANSPOSED on host: weights/output run in [c', i] orientation
    COC = nc.declare_dram_parameter("coc_t", [H, H], f32, isOutput=False)
    TAPS = nc.declare_dram_parameter("taps", [NCHUNK, 128, K * _BAND_W],
                                     f32, isOutput=False)
    # consts columns: 0..3 = horner g3,g2,g1,g0 (poly/a4), 4 = a4,
    # 5.. = -s_k per basis
    CONSTS = nc.declare_dram_parameter("consts", [128, 5 + NUM_BASES_USED],
                                       f32, isOutput=False)
    OUT = nc.declare_dram_parameter("out", [3, H, H], f32, isOutput=True)

    def rearr(ap):  # [512,512] dram view -> [128 part, chunk, col]
        return ap.rearrange("(q p) j -> p q j", p=128)

    with tile.TileContext(nc) as tc:
        import contextlib
        ctx = contextlib.ExitStack()
        with ctx:
            cpool = ctx.enter_context(tc.tile_pool(name="consts", bufs=1))
            tspool = ctx.enter_context(tc.tile_pool(name="tstage", bufs=1))
            tpool = ctx.enter_context(tc.tile_pool(name="ttab", bufs=1))
            wpool = ctx.enter_context(tc.tile_pool(name="weights", bufs=1))
            wtmp = ctx.enter_context(tc.tile_pool(name="wtmp", bufs=3))
            xpool = ctx.enter_context(tc.tile_pool(name="xin", bufs=2))
            xrpool = ctx.enter_context(tc.tile_pool(name="xr", bufs=2))
            apool = ctx.enter_context(tc.tile_pool(name="abig", bufs=5))
            accpool = ctx.enter_context(tc.tile_pool(name="acc", bufs=2))
            mpool = ctx.enter_context(tc.tile_pool(name="mtmp", bufs=3))
            ps1 = ctx.enter_context(
                tc.tile_pool(name="ps", bufs=8, space="PSUM"))
            ps2 = ps1

            consts = cpool.tile([128, 5 + NUM_BASES_USED], f32)
            nc.sync.dma_start(consts[:], CONSTS[:])

            # --- T tables: memset staging, DMA only the diagonal band,
            # round to matmul dtype on DVE ---
            T = []
            for q in range(NCHUNK):
                ts = tspool.tile([128, K * H], f32, tag="ts",
                                 name=f"ts{q}")
                nc.gpsimd.memset(ts[:], 0.0)
                c0 = _BAND_C0[q]
                dst = ts[:].rearrange("p (k j) -> p k j",
                                      k=K)[:, :, c0:c0 + _BAND_W]
                nc.sync.dma_start(dst, TAPS[q].rearrange(
                    "p (k j) -> p k j", k=K))
                tq = tpool.tile([128, K * H], DT, tag=f"T{q}")
                nc.vector.tensor_copy(tq[:], ts[:])
                T.append(tq)

            xrs = {}

            w = []

            def emit_weights():
                # sigma + mixture weights (transposed layout [128, ct, i])
                coc = wtmp.tile([128, K * H], f32, tag="wt", name="coc")
                nc.sync.dma_start(coc[:], rearr(COC[:]))
                # sigma = softplus(w*coc + b) via degree-4 Horner (coeffs
                # from host): q = ((((c+g3)c+g2)c+g1)c+g0)*a4
                sigma = wtmp.tile([128, K * H], f32, tag="wt", name="sigma")
                nc.vector.scalar_tensor_tensor(
                    sigma[:], coc[:], consts[:, 0:1], coc[:],
                    ALU.add, ALU.mult)
                for gi in (1, 2):
                    nc.vector.scalar_tensor_tensor(
                        sigma[:], sigma[:], consts[:, gi:gi + 1],
                        coc[:], ALU.add, ALU.mult)
                nc.vector.tensor_scalar(sigma[:], sigma[:], consts[:, 3:4],
                                        consts[:, 4:5], ALU.add, ALU.mult)
                nc.vector.tensor_scalar_max(sigma[:], sigma[:],
                                            float(SIGMA_MIN))
                nc.vector.tensor_scalar_min(sigma[:], sigma[:],
                                            float(SIGMA_MAX))
                for k in range(K):
                    sq = wtmp.tile([128, K * H], f32, tag="wt",
                                   name=f"sq{k}")
                    nc.scalar.activation(sq[:], sigma[:], AF.Square,
                                         bias=consts[:, 5 + k:6 + k])
                    ek = wpool.tile([128, K * H], f32, tag=f"w{k}")
                    nc.scalar.activation(ek[:], sq[:], AF.Exp, scale=-0.5)
                    w.append(ek)
                t01 = wtmp.tile([128, K * H], f32, tag="wt", name="t01")
                nc.vector.tensor_tensor(t01[:], w[0][:], w[1][:], ALU.add)
                t23 = wtmp.tile([128, K * H], f32, tag="wt", name="t23")
                nc.vector.tensor_tensor(t23[:], w[2][:], w[3][:], ALU.add)
                denom = wtmp.tile([128, K * H], f32, tag="wt", name="denom")
                nc.vector.scalar_tensor_tensor(denom[:], t01[:], float(EPS),
                                               t23[:], ALU.add, ALU.add)
                recip = wtmp.tile([128, K * H], f32, tag="wt", name="recip")
                rscr = wtmp.tile([128, K * H], f32, tag="wt", name="rscr")
                nc.vector.reciprocal_approx_accurate(recip[:], denom[:],
                                                     rscr[:])
                for k in range(K):
                    nc.vector.tensor_tensor(w[k][:], w[k][:], recip[:],
                                            ALU.mult)

            def emit_stage1(ch):
                if ch in xrs:
                    xr = xrs[ch]
                else:
                    xs = xpool.tile([128, K * H], f32, tag="xs",
                                    name=f"xs{ch}")
                    nc.sync.dma_start(xs[:], rearr(IMG[ch]))
                    xr = xrpool.tile([128, K * H], DT, tag="xr",
                                     name=f"xr{ch}")
                    nc.vector.tensor_copy(xr[:], xs[:])
                # stage 1: A^T[c, i] per k, fragments in k-packed windows
                abig = []
                for mt in range(NCHUNK):
                    banks = [ps1.tile([128, 512], f32, tag="ps",
                                      name=f"b1_{ch}_{mt}_{i}")
                             for i in range(len(_BANKS_S1))]
                    # window idx -> (bank tile, offset of segment)
                    seg = {}
                    for b, widxs in zip(banks, _BANKS_S1):
                        off = 0
                        for wi in widxs:
                            seg[wi] = (b, off)
                            off += K * wins1[wi][1]
                    for q in range(NCHUNK):
                        lhsT = xr[:, q * H + 128 * mt: q * H + 128 * mt + 128]
                        for wi, (c0, wd, chunks) in enumerate(wins1):
                            if q not in chunks:
                                continue
                            bank, off = seg[wi]
                            o3 = bank[:, off:off + K * wd].rearrange(
                                "p (k j) -> p k j", k=K)
                            # rhs: cols {k*H + c0 + j, j < wd}
                            rhs = T[q][:].rearrange("p (k j) -> p k j",
                                                    k=K)[:, :, c0:c0 + wd]
                            nc.tensor.matmul(
                                o3, lhsT, rhs,
                                start=(q == chunks[0]),
                                stop=(q == chunks[-1]))
                    ab = apool.tile([128, K * H], DT, tag="ab")
                    abig.append(ab)
                    # drain units: one copy per PSUM bank where the two
                    # packed windows have equal width (f32r: 64+64), else
                    # one copy per window
                    units = []
                    for widxs in _BANKS_S1:
                        if (len(widxs) == 2 and
                                wins1[widxs[0]][1] == wins1[widxs[1]][1]):
                            w0i, w1i = widxs
                            c0, wd, _ = wins1[w0i]
                            bank, off = seg[w0i]
                            src = bank[:, off:off + 2 * K * wd].rearrange(
                                "p (w k j) -> p k w j", w=2, k=K)
                            dst = ab.rearrange(
                                "p (k c) -> p k c",
                                k=K)[:, :, c0:c0 + 2 * wd].rearrange(
                                "p k (w j) -> p k w j", w=2)
                            units.append((src, dst))
                        else:
                            for wi in widxs:
                                c0, wd, _ = wins1[wi]
                                bank, off = seg[wi]
                                src = bank[:, off:off + K * wd].rearrange(
                                    "p (k j) -> p k j", k=K)
                                dst = ab.rearrange(
                                    "p (k j) -> p k j",
                                    k=K)[:, :, c0:c0 + wd]
                                units.append((src, dst))
                    for ui, (src, dst) in enumerate(units):
                        if ui % 2 == 0:
                            nc.scalar.activation(dst, src, AF.Copy)
                        else:
                            nc.vector.tensor_copy(dst, src)
                return abig

            def emit_s2_final(ch, abig):
                # stage 2 (T stationary, A^T moving): Z^T[c', i] into one
                # bank per (k, ct); then weighted accumulation (transposed)
                acc = accpool.tile([128, K * H], f32, tag="acc",
                                   name=f"acc{ch}")
                for k in range(K):
                    for ct in range(NCHUNK):
                        chunks = _S2_CHUNKS[ct]
                        zb = ps2.tile([128, 512], f32, tag="ps")
                        for q2 in chunks:
                            lhsT = T[q2][:, k * H + 128 * ct:
                                         k * H + 128 * ct + 128]
                            rhs = abig[q2][:, k * H:(k + 1) * H]
                            nc.tensor.matmul(
                                zb[:], lhsT, rhs,
                                start=(q2 == chunks[0]),
                                stop=(q2 == chunks[-1]))
                        wsl = w[k][:, ct * 512:(ct + 1) * 512]
                        asl = acc[:, ct * 512:(ct + 1) * 512]
                        if k == 0:
                            nc.vector.tensor_tensor(asl, zb[:], wsl, ALU.mult)
                        else:
                            m = mpool.tile([128, 512], f32, tag="m")
                            nc.vector.tensor_tensor(m[:], zb[:], wsl, ALU.mult)
                            if k == 2:
                                nc.gpsimd.dma_start(asl, m[:],
                                                    accum_op=ALU.add)
                            elif k == 3:
                                # last add on DVE (fast 2x SBUF) so the
                                # per-tile output DMA can fire early
                                nc.vector.tensor_tensor(asl, asl, m[:],
                                                        ALU.add)
                                nc.sync.dma_start(
                                    OUT[ch][128 * ct:128 * (ct + 1), :], asl)
                            else:
                                # k=1 add on DVE too: 450ns vs 1266ns on
                                # gpsimd, shortens the per-tile dep chain
                                nc.vector.tensor_tensor(asl, asl, m[:],
                                                        ALU.add)

            # weights first (ACT-heavy, overlaps stage-1 MM stream)
            emit_weights()
            for ch in range(3):
                emit_s2_final(ch, emit_stage1(ch))

    nc.compile()
    return nc


_PROG = {}


def _get_prog(mode):
    if mode not in _PROG:
        _PROG[mode] = _build(mode)
    return _PROG[mode]


def kernel(image, coc_map, psf_params, w_sigma, b_sigma):
    from concourse.bass_utils import run_bass_kernel_spmd

    B = image.shape[0]
    assert image.shape == (8, 3, H, H)
    nc = _get_prog(MODE)
    taps = _taps_padded()
    _, sigmas = _taps()
    consts = np.empty((128, 5 + NUM_BASES_USED), dtype=np.float32)
    consts[:, :5] = _softplus_poly(
        float(np.asarray(w_sigma).reshape(-1)[0]),
        float(np.asarray(b_sigma).reshape(-1)[0]))[None, :]
    for k in range(NUM_BASES_USED):
        consts[:, 5 + k] = -sigmas[k]
    in_maps = []
    for b in range(B):
        in_maps.append({
            "image": np.ascontiguousarray(image[b], dtype=np.float32),
            "coc_t": np.ascontiguousarray(
                np.asarray(coc_map[b, 0], dtype=np.float32).T),
            "taps": taps,
            "consts": consts,
        })
    res = run_bass_kernel_spmd(nc, in_maps, core_ids=list(range(B)))
    # device output is transposed: [ch, c', i] -> [ch, i, c']
    out = np.stack([res.results[b]["out"] for b in range(B)], axis=0)
    return np.ascontiguousarray(out.transpose(0, 1, 3, 2)).astype(np.float32)


if __name__ == "__main__":
    # smoke: build only
    _get_prog(MODE)
    print("build ok")



# revision 4
# speedup vs baseline: 1.7986x; 1.7986x over previous
"""FFT spatially-variant blur via a rank-4 linear-in-coc factorization.

Reference math: out = sum_k wbar_k(coc) * (psf_k (*) x), with mixture
weights wbar_k over 8 Gaussian PSF bases, sigma = clip(softplus(
0.3*coc + 0.5), 0.2, 12).  With coc in [0,1), sigma lies in
[0.974, 1.172]: only bases 0..3 have non-negligible weight AND the
per-pixel effective kernel field K(c) = sum_k wbar_k(c) psf_k is
linear in c to 5.5e-4 rms:

    K(c) ~= P0 + c * P1          (field fit, rms 5.5e-4)
    P0 ~= l0 u0 u0^T + l1 u1 u1^T    (rank-2, 8e-4)
    P1 ~= m0 w0 w0^T + m1 w1 w1^T    (rank-2, 4e-5)

so the whole module becomes FOUR separable convolutions (two per
plane, accumulated in PSUM) plus a single fused per-pixel mix:

    out = A + coc .* B,   A = P0 (*) x, B = P1 (*) x

Each separable conv is two banded-Toeplitz matmuls on the tensor
engine (bf16 operands, fp32 PSUM):
  stage 1: CC_r^T[j', c] = X^T T1_r  (column conv, r-packed windows)
  stage 2: Z^T[c', c]   += T2_r^T CC_r (row conv, plane-accumulated)
The mix runs as 2 DVE ops per 128x512 tile; there is no per-pixel
weights pipeline at all.  Measured end-to-end rel err ~3.5e-3
(approx 1.3e-3 + bf16 rounding), vs the 2e-2 gate.

Data parallel: core b handles batch sample b (3 channels each).
"""

import numpy as np
import ml_dtypes

PSF_SIZE = 31
SIGMA_MIN = 0.2
SIGMA_MAX = 12.0
EPS = 1e-9
H = 512
NCHUNK = 4   # 512 / 128
R = 4        # separable filters: 0,1 -> plane A; 2,3 -> plane B

# stage-1 windows over the column-conv output: (col0, width,
# [contributing row-chunks]).  Band of chunk q covers cols
# [128q-15, 128q+143); overlap cols get accumulating matmuls from both.
_WINDOWS_S1 = [
    (0, 113, (0,)),
    (113, 30, (0, 1)),
    (143, 98, (1,)),
    (241, 30, (1, 2)),
    (271, 98, (2,)),
    (369, 30, (2, 3)),
    (399, 113, (3,)),
]
# PSUM bank packing (fp32 words per partition <= 512), r-packed widths:
# w0:452 | w1+w3+w5: 3*120=360 | w2:392 | w4:392 | w6:452
_BANKS_S1 = [[0], [1, 3, 5], [2], [4], [6]]

# stage 2: per output-column tile ct, contraction over band chunks
_S2_CHUNKS = [tuple(q for q in (ct - 1, ct, ct + 1) if 0 <= q < NCHUNK)
              for ct in range(NCHUNK)]


def _filters(ws, bs):
    """Rank-4 linear-in-c factorization of the kernel field.

    Returns (t1_taps[4][31], t2_taps[4][31]) fp64; filter r contributes
    outer(t1[r], t2[r]) to plane A (r<2) or plane B (r>=2)."""
    lo = (-PSF_SIZE) // 2
    hi = PSF_SIZE // 2
    x = np.linspace(lo, hi, PSF_SIZE, dtype=np.float32).astype(np.float64)
    gx, gy = np.meshgrid(x, x, indexing='ij')
    sigmas = np.linspace(SIGMA_MIN, SIGMA_MAX, 8, dtype=np.float32)
    sigmas = sigmas.astype(np.float64)
    psfs = []
    for s in sigmas:
        g = np.exp(-(gx ** 2 + gy ** 2) / (2.0 * s * s + EPS))
        psfs.append(g / (g.sum() + EPS))
    psfs = np.array(psfs).reshape(8, -1)

    cg = np.linspace(0.0, 1.0, 4001)
    sig = np.clip(np.logaddexp(0.0, ws * cg + bs), SIGMA_MIN, SIGMA_MAX)
    w = np.exp(-(sig[:, None] - sigmas[None, :]) ** 2 / 2.0)
    w = w / (w.sum(1, keepdims=True) + EPS)
    M = w @ psfs                                     # [nc, 961]
    V = np.vander(cg, 2, increasing=True)            # [nc, 2]
    coef, *_ = np.linalg.lstsq(V, M, rcond=None)
    t1, t2 = [], []
    for m in range(2):
        P = coef[m].reshape(PSF_SIZE, PSF_SIZE)
        evals, evecs = np.linalg.eigh(P)
        idx = np.argsort(-np.abs(evals))[:2]
        for i in idx:
            t1.append(evecs[:, i])
            t2.append(evals[i] * evecs[:, i])
    return t1, t2


def _toeplitz_tables(taps_list):
    """[4 (q), 128, R*512] bf16: T[q][p, r*512+c] = taps_r[15+c-(128q+p)]
    inside the band, else 0."""
    tab = np.zeros((NCHUNK, 128, R * H), dtype=np.float64)
    for r, taps in enumerate(taps_list):
        for q in range(NCHUNK):
            for p in range(128):
                row = 128 * q + p
                j0 = max(0, row - 15)
                j1 = min(H, row + 16)
                tab[q, p, r * H + j0:r * H + j1] = \
                    taps[15 + np.arange(j0, j1) - row]
    return tab.astype(ml_dtypes.bfloat16)


def _build():
    import concourse.bass as bass  # noqa: F401
    import concourse.tile as tile
    from concourse import mybir, bacc

    f32 = mybir.dt.float32
    bf16 = mybir.dt.bfloat16
    AF = mybir.ActivationFunctionType
    ALU = mybir.AluOpType
    wins = _WINDOWS_S1

    nc = bacc.Bacc("TRN2", target_bir_lowering=False, debug=False,
                   disable_frame_to_traceback=True)
    IMG = nc.declare_dram_parameter("image", [3, H, H], bf16, isOutput=False)
    # coc TRANSPOSED on host: mix/output run in [c', c] orientation
    COC = nc.declare_dram_parameter("coc_t", [H, H], bf16, isOutput=False)
    T1 = nc.declare_dram_parameter("t1", [NCHUNK, 128, R * H], bf16,
                                   isOutput=False)
    T2 = nc.declare_dram_parameter("t2", [NCHUNK, 128, R * H], bf16,
                                   isOutput=False)
    OUT = nc.declare_dram_parameter("out", [3, H, H], bf16, isOutput=True)

    def rearr(ap):  # [512,512] dram view -> [128 part, chunk, col]
        return ap.rearrange("(q p) j -> p q j", p=128)

    with tile.TileContext(nc) as tc:
        import contextlib
        ctx = contextlib.ExitStack()
        with ctx:
            tpool = ctx.enter_context(tc.tile_pool(name="ttab", bufs=1))
            cpool = ctx.enter_context(tc.tile_pool(name="coc", bufs=1))
            xpool = ctx.enter_context(tc.tile_pool(name="xin", bufs=1))
            apool = ctx.enter_context(tc.tile_pool(name="abig", bufs=8))
            mpool = ctx.enter_context(tc.tile_pool(name="mtmp", bufs=3))
            accpool = ctx.enter_context(tc.tile_pool(name="acc", bufs=3))
            ps = ctx.enter_context(
                tc.tile_pool(name="ps", bufs=8, space="PSUM"))

            # persistent inputs
            t1 = []
            t2 = []
            for q in range(NCHUNK):
                a = tpool.tile([128, R * H], bf16, tag=f"t1_{q}")
                nc.sync.dma_start(a[:], T1[q])
                t1.append(a)
            coc = cpool.tile([128, NCHUNK * H], bf16, tag="coc")
            nc.sync.dma_start(coc[:], rearr(COC[:]))
            xs = []
            for ch in range(3):
                x = xpool.tile([128, NCHUNK * H], bf16, tag=f"xs{ch}")
                nc.sync.dma_start(x[:], rearr(IMG[ch]))
                xs.append(x)
            for q in range(NCHUNK):
                a = tpool.tile([128, R * H], bf16, tag=f"t2_{q}")
                nc.sync.dma_start(a[:], T2[q])
                t2.append(a)

            drain_rr = [0]

            def emit_stage1(ch):
                """Column conv: abig[mt][p=j', r, c] = CC_r[c, 128mt+p]."""
                xr = xs[ch]
                abig = []
                for mt in range(NCHUNK):
                    banks = [ps.tile([128, 512], f32, tag="ps",
                                     name=f"b1_{ch}_{mt}_{i}")
                             for i in range(len(_BANKS_S1))]
                    seg = {}
                    for b, widxs in zip(banks, _BANKS_S1):
                        off = 0
                        for wi in widxs:
                            seg[wi] = (b, off)
                            off += R * wins[wi][1]
                    for q in range(NCHUNK):
                        lhsT = xr[:, q * H + 128 * mt: q * H + 128 * mt + 128]
                        for wi, (c0, wd, chunks) in enumerate(wins):
                            if q not in chunks:
                                continue
                            bank, off = seg[wi]
                            o3 = bank[:, off:off + R * wd].rearrange(
                                "p (r j) -> p r j", r=R)
                            rhs = t1[q][:].rearrange(
                                "p (r j) -> p r j", r=R)[:, :, c0:c0 + wd]
                            nc.tensor.matmul(
                                o3, lhsT, rhs,
                                start=(q == chunks[0]),
                                stop=(q == chunks[-1]))
                    ab = apool.tile([128, R * H], bf16, tag="ab",
                                    name=f"ab{ch}_{mt}")
                    abig.append(ab)
                    # drains: one copy per bank; the 3x30 bank maps onto the
                    # three 30-wide windows (c = 128w + j, j in [113,143))
                    units = []
                    b, _ = seg[0]
                    units.append((
                        b[:, :R * 113].rearrange("p (r j) -> p r j", r=R),
                        ab.rearrange("p (r c) -> p r c", r=R)[:, :, 0:113]))
                    b, _ = seg[1]
                    for w in range(3):
                        units.append((
                            b[:, :3 * R * 30].rearrange(
                                "p (w r j) -> p w r j", w=3, r=R)[:, w],
                            ab.rearrange("p (r c) -> p r c", r=R)
                            [:, :, 113 + 128 * w:143 + 128 * w]))
                    for wi, c0, wd in ((2, 143, 98), (4, 271, 98),
                                       (6, 399, 113)):
                        b, _ = seg[wi]
                        units.append((
                            b[:, :R * wd].rearrange("p (r j) -> p r j", r=R),
                            ab.rearrange("p (r c) -> p r c",
                                         r=R)[:, :, c0:c0 + wd]))
                    # PSUM is only reachable from ACT/DVE (GpSimd cannot
                    # touch it); DVE also carries the mix, so it gets only
                    # the first unit per tile
                    for ui, (src, dst) in enumerate(units):
                        if ui == 0:
                            nc.vector.tensor_copy(dst, src)
                        else:
                            nc.scalar.activation(dst, src, AF.Copy)
                return abig

            def emit_s2_mix(ch, abig):
                """Row conv accumulated per plane + fused linear mix."""
                for ct in range(NCHUNK):
                    chunks = _S2_CHUNKS[ct]
                    planes = []
                    for pl in range(2):
                        zb = ps.tile([128, 512], f32, tag="ps",
                                     name=f"z{pl}_{ch}_{ct}")
                        rs = (0, 1) if pl == 0 else (2, 3)
                        first = (rs[0], chunks[0])
                        last = (rs[-1], chunks[-1])
                        for r in rs:
                            for q2 in chunks:
                                lhsT = t2[q2][:, r * H + 128 * ct:
                                              r * H + 128 * ct + 128]
                                rhs = abig[q2][:, r * H:(r + 1) * H]
                                nc.tensor.matmul(
                                    zb[:], lhsT, rhs,
                                    start=((r, q2) == first),
                                    stop=((r, q2) == last))
                        planes.append(zb)
                    csl = coc[:, ct * H:(ct + 1) * H]
                    m = mpool.tile([128, 512], bf16, tag="m")
                    nc.vector.tensor_tensor(m[:], planes[1][:], csl, ALU.mult)
                    acc = accpool.tile([128, 512], bf16, tag="acc")
                    nc.vector.tensor_tensor(acc[:], planes[0][:], m[:],
                                            ALU.add)
                    nc.sync.dma_start(
                        OUT[ch][128 * ct:128 * (ct + 1), :], acc[:])

            # software pipeline: keep the PE busy with ch+1 stage 1 while
            # ch's drains/mix run on ACT/DVE/Pool
            ab0 = emit_stage1(0)
            ab1 = emit_stage1(1)
            emit_s2_mix(0, ab0)
            ab2 = emit_stage1(2)
            emit_s2_mix(1, ab1)
            emit_s2_mix(2, ab2)

    nc.compile()
    return nc


_PROG = None


def _get_prog():
    global _PROG
    if _PROG is None:
        _PROG = _build()
    return _PROG


_TABLES = {}


def _get_tables(ws, bs):
    key = (float(ws), float(bs))
    if key not in _TABLES:
        t1, t2 = _filters(*key)
        _TABLES[key] = (_toeplitz_tables(t1), _toeplitz_tables(t2))
    return _TABLES[key]


def kernel(image, coc_map, psf_params, w_sigma, b_sigma):
    from concourse.bass_utils import run_bass_kernel_spmd

    B = image.shape[0]
    assert image.shape == (8, 3, H, H)
    nc = _get_prog()
    tab1, tab2 = _get_tables(
        float(np.asarray(w_sigma).reshape(-1)[0]),
        float(np.asarray(b_sigma).reshape(-1)[0]))
    bf = ml_dtypes.bfloat16
    image = np.asarray(image)
    coc_map = np.asarray(coc_map)
    in_maps = []
    for b in range(B):
        in_maps.append({
            "image": np.ascontiguousarray(image[b].astype(bf)),
            "coc_t": np.ascontiguousarray(coc_map[b, 0].T.astype(bf)),
            "t1": tab1,
            "t2": tab2,
        })
    res = run_bass_kernel_spmd(nc, in_maps, core_ids=list(range(B)))
    # device output is transposed: [ch, c', c] -> [ch, c, c']
    out = np.stack([res.results[b]["out"] for b in range(B)], axis=0)
    return np.ascontiguousarray(
        out.transpose(0, 1, 3, 2)).astype(np.float32)


if __name__ == "__main__":
    _get_prog()
    print("build ok")
